# revision 60
# baseline (speedup 1.0000x reference)
"""Causal self-attention (B=4, T=2048, C=1024, H=16) on 8 TRN2 NeuronCores.

Sharding: core c -> batch b = c//2, head-group g2 = c%2 (8 heads, feature
columns j0 = g2*512 .. +512).  Megatron column-parallel QKV + row-parallel
out-projection; host sums the two partials per batch.  No collectives.

Single fused phase, column-pipelined so the Activation engine's exp work
(~147us, the #2 engine) overlaps the entire PE timeline:
  - QKV + out-projection GEMMs in fp8e4m3 DoubleRow (K=256, 0.5 cyc/row)
    with 3-term hi/lo residual compensation (x=xh+xl, w=wh+wl; the lo*lo
    term is dropped) -> bf16-class accuracy at ~0.75x bf16's matmul cost.
    Host pre-scales x by 8 and w by 64 to keep e4m3 in its normal range;
    the psum->SBUF copy divides the 2^9 back out.
  - QK in bf16 (1 cyc/row at any N; K=64 per head): attT [kpos, q] per
    (head-pair, ki) in a 2-bank psum tile, diagonal tiles narrowed to the
    causal columns; exp on ScalarE -> bf16 ae; staircase zeroed by gpsimd.
  - AV reversed: lhsT = ae 128-q chunk, rhs = v|ones [128,65] -> y natural
    [q, 65] in psum, M=128 (65 cyc vs 512 per k-tile/head), rowsum in col
    64.  All 8 sub-chunk accumulations of a block form ONE psum group per
    bank (start only on the bank's first matmul: its bank-wide pending-
    zero zero-initializes each chunk's first write; interleaved per-chunk
    groups would wipe sibling partials).  Partials accumulate across
    columns in an SBUF fp32 y_acc via DVE.
  - finalize per diagonal block: reciprocal_approx of the rowsums,
    per-partition normalize (x8 for fp8) -> bf16, PE-transpose, then an
    fp8 hi/lo split on DVE feeding the DoubleRow out-projection.
  - projection/out-projection chains are emitted as deferred generators
    (q/k/v at column flush points, out-proj at end of emission); the tile
    scheduler hoists them by dependency to fill the QK->exp->AV pipeline
    gaps.  DMA descriptor generation costs ~625ns per transfer and all
    copies serialize on one ~330 GB/s stream, so transfers are few and
    strictly first-need-first (g=0 q/k hi slices + x col 0 lead).
  - output is written bf16 and the partial sums are combined in fp32 on
    the host.
"""
import numpy as np

B, T, C = 4, 2048, 1024
NC = 8
P = 128
CO = 8           # c-tiles of 128 (contraction for QKV)
QB = 512         # q/k column block
NCOL = T // QB   # 4
D = 64           # head dim
W65 = 65         # [v | ones]

_CACHE = {}

CFG = {"attb": 2, "phb": 2, "ppb": 2, "aeb": 16, "obb": 6, "ynb": 2,
       "fill": [0, 0, 0, 3], "dfill": 2, "lag": 3, "vterms": 3, "pterms": 3}


def _build():
    from contextlib import ExitStack
    import concourse.bass as bass
    import concourse.tile as tile
    from concourse import bacc, mybir

    F32 = mybir.dt.float32
    BF16 = mybir.dt.bfloat16
    AF = mybir.ActivationFunctionType
    ADD = mybir.AluOpType.add

    F8 = mybir.dt.float8e4
    DR = mybir.MatmulPerfMode.DoubleRow

    nc = bacc.Bacc("TRN2", target_bir_lowering=False, debug=False,
                   dynamic_dma_scratch_size=2048)
    dr_in = {}
    for nm in ("xh", "xl"):
        dr_in[nm] = nc.dram_tensor(nm, [C, T], F8, kind="ExternalInput").ap()
    for nm in ("wqh", "wql", "wkh", "wkl", "wvh", "wvl"):
        dr_in[nm] = nc.dram_tensor(nm, [C, QB], F8, kind="ExternalInput").ap()
    for nm in ("wph", "wpl"):
        dr_in[nm] = nc.dram_tensor(nm, [QB, C], F8, kind="ExternalInput").ap()
    idn = nc.dram_tensor("idn", [P, P], BF16, kind="ExternalInput").ap()
    out = nc.dram_tensor("out", [T, C], BF16, kind="ExternalOutput").ap()

    xh3 = dr_in["xh"].rearrange("(co ci) t -> ci co t", ci=P)  # [128,8,2048]
    xl3 = dr_in["xl"].rearrange("(co ci) t -> ci co t", ci=P)
    w3 = {nm: dr_in[nm].rearrange("(co ci) j -> ci co j", ci=P)
          for nm in ("wqh", "wql", "wkh", "wkl", "wvh", "wvl")}
    wp3 = {nm: dr_in[nm].rearrange("(go gi) m -> gi go m", gi=P)
           for nm in ("wph", "wpl")}                            # [128,4,1024]

    # host scales: x *= 8, w *= 64 (keeps e4m3 operands in normal range);
    # psum results carry 2^9, descaled in the psum->SBUF copy
    DESC = 1.0 / 512.0

    with tile.TileContext(nc) as tc, ExitStack() as ctx:
        persist = ctx.enter_context(tc.tile_pool(name="persist", bufs=1))
        xhs = persist.tile([P, CO, T], F8, tag="xhs", name="xhs")
        xls = persist.tile([P, CO, T], F8, tag="xls", name="xls")
        ws = {nm: persist.tile([P, CO, QB], F8, tag=nm, name=nm)
              for nm in ("wqh", "wql", "wkh", "wkl", "wvh", "wvl")}
        wps = {nm: persist.tile([P, 4, C], F8, tag=nm, name=nm)
               for nm in ("wph", "wpl")}
        qts = [persist.tile([P, T], BF16, tag=f"qt{g}", name=f"qt{g}")
               for g in range(4)]
        kts = [persist.tile([P, T], BF16, tag=f"kt{g}", name=f"kt{g}")
               for g in range(4)]
        vts = persist.tile([P, 16, CO, W65], BF16, tag="vts", name="vts")
        # y accumulator: [q 128, block 16, head2, qchunk 4, v|rowsum 65]
        yac = persist.tile([P, 16, 2, 4, W65], F32, tag="yac", name="yac")
        yTh = persist.tile([P, 4, T], F8, tag="yTh", name="yTh")
        yTl = persist.tile([P, 4, T], F8, tag="yTl", name="yTl")
        idt = persist.tile([P, P], BF16, tag="idt", name="idt")

        # ---- input DMA: descriptor generation costs ~625ns per dma_start
        # and copies serialize at ~330 GB/s, so use FEW transfers ordered
        # by first need: g=0 q/k hi slices + x_hi col 0 (gates the first
        # exp), the lo twins, wv (gates the first AVs), the rest.
        nc.sync.dma_start(ws["wqh"][:, :, 0:P], w3["wqh"][:, :, 0:P])
        nc.sync.dma_start(xhs[:, 0:4, 0:QB], xh3[:, 0:4, 0:QB])
        nc.sync.dma_start(xhs[:, 4:CO, 0:QB], xh3[:, 4:CO, 0:QB])
        nc.sync.dma_start(ws["wkh"][:, :, 0:P], w3["wkh"][:, :, 0:P])
        nc.sync.dma_start(xls[:, :, 0:QB], xl3[:, :, 0:QB])
        nc.sync.dma_start(ws["wql"][:, :, 0:P], w3["wql"][:, :, 0:P])
        nc.sync.dma_start(ws["wkl"][:, :, 0:P], w3["wkl"][:, :, 0:P])
        nc.sync.dma_start(ws["wvh"][:], w3["wvh"])
        nc.sync.dma_start(ws["wvl"][:], w3["wvl"])
        for g in range(1, 4):
            for nm in ("wqh", "wkh", "wql", "wkl"):
                nc.sync.dma_start(ws[nm][:, :, g * P:(g + 1) * P],
                                  w3[nm][:, :, g * P:(g + 1) * P])
            if g == 1:
                nc.sync.dma_start(idt[:], idn)
                for xs_, x3_ in ((xhs, xh3), (xls, xl3)):
                    nc.sync.dma_start(xs_[:, :, QB:2 * QB],
                                      x3_[:, :, QB:2 * QB])
        nc.sync.dma_start(wps["wph"][:], wp3["wph"])
        nc.sync.dma_start(wps["wpl"][:], wp3["wpl"])
        for cc in range(2, NCOL):
            for xs_, x3_ in ((xhs, xh3), (xls, xl3)):
                nc.sync.dma_start(xs_[:, :, cc * QB:(cc + 1) * QB],
                                  x3_[:, :, cc * QB:(cc + 1) * QB])
        nc.vector.memset(vts[:, :, :, D:W65], 1.0)

        pp = ctx.enter_context(
            tc.tile_pool(name="pp", bufs=CFG["ppb"], space="PSUM"))
        att = ctx.enter_context(
            tc.tile_pool(name="att", bufs=CFG["attb"], space="PSUM"))
        php = ctx.enter_context(
            tc.tile_pool(name="php", bufs=CFG["phb"], space="PSUM"))
        aep = ctx.enter_context(tc.tile_pool(name="aep", bufs=CFG["aeb"]))
        ynp = ctx.enter_context(tc.tile_pool(name="ynp", bufs=CFG["ynb"]))
        rcp = ctx.enter_context(tc.tile_pool(name="rcp", bufs=2))
        obp = ctx.enter_context(tc.tile_pool(name="obp", bufs=CFG["obb"]))

        # ---------- filler machinery: chains as generators ----
        # fp8 DoubleRow, K=256 per matmul; x = xh+xl, w = wh+wl; the three
        # compensation terms (hh, hl, lh) are emitted term-major so the lo
        # operands are only needed 4 matmuls into the chain.
        def proj_chain(dst_kind, g_or_ki, cc):
            """One [128,512] projection chain (12 DR matmuls + copy)."""
            ps = pp.tile([P, QB], F32, tag="pp", name="pp")
            if dst_kind in ("q", "k"):
                g = g_or_ki
                wh, wl = (("wqh", "wql") if dst_kind == "q"
                          else ("wkh", "wkl"))
                terms = [(ws[wh], xhs), (ws[wh], xls), (ws[wl], xhs)]
                for ti, (w_t, x_t) in enumerate(terms):
                    for j in range(4):
                        nc.tensor.matmul(
                            ps[:], w_t[:, 2 * j:2 * j + 2, g * P:(g + 1) * P],
                            x_t[:, 2 * j:2 * j + 2, cc * QB:(cc + 1) * QB],
                            start=(ti == 0 and j == 0),
                            stop=(ti == 2 and j == 3), perf_mode=DR)
                        yield
                dst = qts[g] if dst_kind == "q" else kts[g]
                nc.vector.tensor_scalar_mul(
                    dst[:, cc * QB:(cc + 1) * QB], ps[:], DESC)
            else:
                ki = g_or_ki
                terms = [(xhs, "wvh"), (xhs, "wvl"), (xls, "wvh")]
                terms = terms[:CFG["vterms"]]
                nt = len(terms)
                for ti, (x_t, wnm) in enumerate(terms):
                    for j in range(4):
                        nc.tensor.matmul(
                            ps[:], x_t[:, 2 * j:2 * j + 2, ki * P:(ki + 1) * P],
                            ws[wnm][:, 2 * j:2 * j + 2, :],
                            start=(ti == 0 and j == 0),
                            stop=(ti == nt - 1 and j == 3), perf_mode=DR)
                        yield
                nc.vector.tensor_scalar_mul(
                    vts[:, ki, :, 0:D],
                    ps[:].rearrange("p (h d) -> p h d", d=D), DESC)
            yield

        def po_chain(qb, tt, mh, merged=False):
            """Out-projection chain for one [128 t, 512 m] tile; merged=True
            does both m-halves in one chain with a single [128,1024] DMA
            (fewer descriptor slots on the tail's critical drain)."""
            t0 = qb * QB + tt * P
            terms = [(yTh, "wph"), (yTh, "wpl"), (yTl, "wph")]
            terms = terms[:CFG["pterms"]]
            nt = len(terms)
            mhs = (0, 1) if merged else (mh,)
            ob2 = (obp.tile([P, C], BF16, tag="ob2", name="ob2")
                   if merged else None)
            for m in mhs:
                ps = pp.tile([P, QB], F32, tag="pp", name="pp")
                for ti, (y_t, wnm) in enumerate(terms):
                    for jc in range(2):
                        nc.tensor.matmul(
                            ps[:], y_t[:, 2 * jc:2 * jc + 2, t0:t0 + P],
                            wps[wnm][:, 2 * jc:2 * jc + 2,
                                     m * QB:(m + 1) * QB],
                            start=(ti == 0 and jc == 0),
                            stop=(ti == nt - 1 and jc == 1), perf_mode=DR)
                        yield
                if merged:
                    nc.vector.tensor_scalar_mul(
                        ob2[:, m * QB:(m + 1) * QB], ps[:], DESC)
                else:
                    ob = obp.tile([P, QB], BF16, tag="ob", name="ob")
                    nc.vector.tensor_scalar_mul(ob[:], ps[:], DESC)
                    nc.sync.dma_start(
                        out[t0:t0 + P, m * QB:(m + 1) * QB], ob[:])
            if merged:
                nc.sync.dma_start(out[t0:t0 + P, :], ob2[:])
            yield

        class Weaver:
            """Two-priority FIFO of labelled generators; emits filler steps
            between attention units.  Deadline work (q/k/v projections) sits
            in the main queue; out-projections sit in the low queue and
            automatically fill thin regions (diagonal groups, tail).
            gens[0] may be mid-chain (holds a psum buf) so it is always
            finished before out-of-order label flushes."""
            def __init__(self):
                self.gens = []   # [label, gen, started]
                self.low = []

            def push(self, gen, label=None):
                self.gens.append([label, gen, False])

            def push_low(self, gen):
                self.low.append([None, gen, False])

            def _step_q(self, q):
                ent = q[0]
                ent[2] = True
                try:
                    next(ent[1])
                    return True
                except StopIteration:
                    q.pop(0)
                    return False

            def step(self, n):
                while n > 0:
                    if self.gens:
                        if self._step_q(self.gens):
                            n -= 1
                    elif self.low:
                        if self._step_q(self.low):
                            n -= 1
                    else:
                        return

            def _finish_head(self):
                if self.gens and self.gens[0][2]:
                    ent = self.gens[0]
                    for _ in ent[1]:
                        pass
                    self.gens.pop(0)

            def flush(self, label=None):
                if label is None:
                    while self.gens or self.low:
                        self.step(1 << 30)
                    return
                if not any(e[0] == label for e in self.gens):
                    return
                self._finish_head()
                rest = []
                for ent in self.gens:
                    if ent[0] == label:
                        for _ in ent[1]:
                            pass
                    else:
                        rest.append(ent)
                self.gens = rest

        wv_ = Weaver()
        MUL = mybir.AluOpType.mult
        LAG = CFG["lag"]
        FILLS = CFG["fill"]

        def finalize(g, qb, on_act=False):
            """Block (g, qb) complete in yac: recip, normalize (scaled x8
            for the fp8 out-projection), hi/lo split, transpose both.
            on_act: run the normalize on the Activation engine (idle at the
            kernel tail) to shorten the DVE critical path into po(3)."""
            blk = g * 4 + qb
            rc = rcp.tile([P, 8], F32, tag="rc", name="rc")
            nc.vector.reciprocal_approx_fast(
                rc[:], yac[:, blk, :, :, D].rearrange("p a b -> p (a b)"))
            yn = ynp.tile([P, 4, P], BF16, tag="yn", name="yn")
            if on_act:
                rc8 = rcp.tile([P, 8], F32, tag="rc8", name="rc8")
                nc.vector.tensor_scalar_mul(rc8[:], rc[:], 8.0)
            for h2 in range(2):
                for qt in range(4):
                    if on_act:
                        nc.scalar.activation(
                            yn[:, qt, h2 * D:(h2 + 1) * D],
                            yac[:, blk, h2, qt, 0:D],
                            AF.Copy,
                            scale=rc8[:, h2 * 4 + qt:h2 * 4 + qt + 1])
                    else:
                        nc.vector.tensor_scalar(
                            yn[:, qt, h2 * D:(h2 + 1) * D],
                            yac[:, blk, h2, qt, 0:D],
                            rc[:, h2 * 4 + qt:h2 * 4 + qt + 1], 8.0,
                            MUL, MUL)
            tp = php.tile([P, 4, P], BF16, tag="ph", name="tp")
            for qt in range(4):
                nc.tensor.transpose(tp[:, qt, :], yn[:, qt, :], idt[:])
            # hi/lo fp8 split AFTER the transpose (fp8 PE transpose needs
            # stride-2 psum outputs, so split on DVE instead)
            yh_dst = yTh[:, g, qb * QB:(qb + 1) * QB]
            nc.vector.tensor_copy(
                yh_dst, tp[:].rearrange("p a b -> p (a b)"))
            nc.vector.tensor_tensor(
                yTl[:, g, qb * QB:(qb + 1) * QB],
                tp[:].rearrange("p a b -> p (a b)"), yh_dst,
                mybir.AluOpType.subtract)

        # ---------------- unified unit stream ----------------
        ph_tiles = {}       # (g, qb) -> [ph_h0, ph_h1]
        fin_count = {}      # column -> finalizes completed

        def emit_av(job):
            cc, g, qb, tt, ae, diag = job
            ki = cc * 4 + tt
            key = (g, qb)
            if tt == 0:
                ph_tiles[key] = [
                    php.tile([P, 4, W65], F32, tag="ph", name="ph")
                    for _ in range(2)]
            ph = ph_tiles[key]
            # ONE accumulation group per head-tile (= per psum bank) for the
            # whole block: start only on the bank's first matmul (its
            # bank-wide pending-zero zero-initializes every chunk's first
            # write), stop only on the last.  Interleaved per-chunk groups
            # would clobber each other: start re-marks the WHOLE 2KB zero
            # region, wiping sibling chunks' partials.
            for h2 in range(2):
                h = 2 * g + h2
                for qt in range(4):
                    if diag and qt < tt:
                        continue
                    nc.tensor.matmul(
                        ph[h2][:, qt, :],
                        ae[:, h2, qt * P:(qt + 1) * P],
                        vts[:, ki, h, 0:W65],
                        start=(tt == 0 and qt == 0),
                        stop=(tt == 3 and qt == 3))
            if tt == 3:
                blk = g * 4 + qb
                for h2 in range(2):
                    if cc == 0:
                        nc.vector.tensor_copy(yac[:, blk, h2], ph[h2][:])
                    else:
                        nc.vector.tensor_tensor(
                            yac[:, blk, h2], ph[h2][:], yac[:, blk, h2], ADD)
                del ph_tiles[key]
                if diag:
                    finalize(g, qb, on_act=False)
                    fin_count[cc] = fin_count.get(cc, 0) + 1
                    if fin_count[cc] == 4:
                        for tt2 in range(4):
                            for mh in range(2):
                                wv_.push_low(po_chain(qb, tt2, mh))

        # seed: q/k for (g=0, column 0) first (DMA-paced), v chains after
        wv_.push(proj_chain("q", 0, 0))
        wv_.push(proj_chain("k", 0, 0))
        wv_.flush()
        for tt in range(4):
            wv_.push(proj_chain("v", tt, 0), ("kv", 0))
        qpushed = {0}

        pend = []
        for cc in range(NCOL):
            FILL = FILLS[cc]
            if cc > 0:
                wv_.flush(("kv", cc))  # k/v of this column must be resident
            for qb in range(cc, NCOL):
                wv_.flush(("q", qb))  # q rows for this group
                if qb + 1 < NCOL and qb + 1 not in qpushed:
                    for g in range(4):
                        wv_.push(proj_chain("q", g, qb + 1), ("q", qb + 1))
                    qpushed.add(qb + 1)
                for g in range(4):
                    if cc == 0 and qb == 0 and g > 0:
                        # JIT q/k for the remaining head-pairs of column 0
                        wv_.push(proj_chain("q", g, 0))
                        wv_.push(proj_chain("k", g, 0))
                        wv_.flush()
                    for tt in range(4):
                        ki = cc * 4 + tt
                        diag = (qb == cc)
                        dq = tt * P if diag else 0
                        q0 = qb * QB
                        ap = att.tile([P, 2, QB], F32, tag="att", name="ap")
                        for h2 in range(2):
                            rows = slice(h2 * D, h2 * D + D)
                            nc.tensor.matmul(
                                ap[:, h2, dq:QB],
                                kts[g][rows, ki * P:(ki + 1) * P],
                                qts[g][rows, q0 + dq:q0 + QB],
                                start=True, stop=True)
                        ae = aep.tile([P, 2, QB], BF16, tag="ae", name="ae")
                        nc.scalar.activation(
                            ae[:, :, dq:QB], ap[:, :, dq:QB], AF.Exp)
                        if diag:
                            for h2 in range(2):
                                nc.gpsimd.affine_select(
                                    out=ae[:, h2, dq:dq + P],
                                    in_=ae[:, h2, dq:dq + P],
                                    compare_op=mybir.AluOpType.is_ge,
                                    fill=0.0, base=0,
                                    pattern=[[1, P]], channel_multiplier=-1)
                        pend.append((cc, g, qb, tt, ae, diag))
                        # the very first block keeps all AVs pending until
                        # its v projections have been emitted
                        lag = (4 if (cc == 0 and qb == 0 and g == 0)
                               else (2 if cc == NCOL - 1 else LAG))
                        if len(pend) > lag:
                            emit_av(pend.pop(0))
                        wv_.step(FILL + (CFG["dfill"] if diag else 0))
                    if cc == 0 and qb == 0 and g == 0:
                        wv_.flush(("kv", 0))
            # next column's k/v seeds weave into the remaining stream
            if cc + 1 < NCOL:
                for g in range(4):
                    wv_.push(proj_chain("k", g, cc + 1), ("kv", cc + 1))
                for tt in range(4):
                    wv_.push(proj_chain("v", (cc + 1) * 4 + tt, cc + 1),
                             ("kv", cc + 1))
        while pend:
            emit_av(pend.pop(0))
            wv_.step(FILLS[-1])

        wv_.flush()

    nc.finalize()
    return nc


def _hl(a, f8):
    """Split fp32 array into fp8 hi + lo (residual) parts."""
    hi = a.astype(f8)
    lo = (a - hi.astype(np.float32)).astype(f8)
    return hi, lo


def _prep_inputs(x, Wq, Wk, Wv, Wp):
    import math
    import ml_dtypes
    bf16 = ml_dtypes.bfloat16
    f8 = ml_dtypes.float8_e4m3
    scale = 1.0 / math.sqrt(D)
    XS, WS = 8.0, 64.0
    idn = np.eye(P, dtype=bf16)
    in_maps = []
    for c in range(NC):
        b, g2 = c // 2, c % 2
        j0 = g2 * QB
        xh, xl = _hl(np.ascontiguousarray(x[b].T) * XS, f8)
        wqh, wql = _hl(np.ascontiguousarray((Wq[j0:j0 + QB] * scale).T) * WS,
                       f8)
        wkh, wkl = _hl(np.ascontiguousarray(Wk[j0:j0 + QB].T) * WS, f8)
        wvh, wvl = _hl(np.ascontiguousarray(Wv[j0:j0 + QB].T) * WS, f8)
        wph, wpl = _hl(np.ascontiguousarray(Wp[:, j0:j0 + QB].T) * WS, f8)
        in_maps.append({
            "xh": xh, "xl": xl, "wqh": wqh, "wql": wql,
            "wkh": wkh, "wkl": wkl, "wvh": wvh, "wvl": wvl,
            "wph": wph, "wpl": wpl, "idn": idn,
        })
    return in_maps


def kernel(x, Wq, Wk, Wv, Wp, _trace=False):
    from concourse.bass_utils import run_bass_kernel_spmd

    x = np.asarray(x); Wq = np.asarray(Wq); Wk = np.asarray(Wk)
    Wv = np.asarray(Wv); Wp = np.asarray(Wp)

    if "nc" not in _CACHE:
        _CACHE["nc"] = _build()
    nc = _CACHE["nc"]

    in_maps = _prep_inputs(x, Wq, Wk, Wv, Wp)
    res = run_bass_kernel_spmd(nc, in_maps, core_ids=list(range(NC)),
                               trace=_trace)
    outs = [r["out"] for r in res.results]
    full = np.empty((B, T, C), np.float32)
    for b in range(B):
        full[b] = (outs[2 * b].astype(np.float32)
                   + outs[2 * b + 1].astype(np.float32))
    if _trace:
        _CACHE["last_results"] = res
    return full
